# revision 23
# baseline (speedup 1.0000x reference)
"""Gated multi-head attention (AlphaFold-style) on 8 Trainium2 NeuronCores.

Sharding: 8 cores = 2 batches x 4 query-chunks of 512 rows; each core does all
8 heads for its (b, q-chunk); outputs are disjoint row blocks (no collectives).

Per-core pipeline:
 - q/k projected to fp8e4m3, DoubleRow-packed: partition 32g+dd holds head
   2g+t dim dd on k-tile t.  kT8 carries both heads of a pair; the q side is
   split into qT8e/qT8o with zeros in the partner head's k-tile slots, so one
   DR matmul (0.5 cyc/row) contracts exactly one head's 32 dims and emits
   s_scaled = A16*s (A16 = 128*log2(e) folded into Wq host-side).
 - softmax weights w = exp(s+b) as bf16 BIT PATTERNS, three routes balancing
   ACT/DVE/Pool:
     S: one DVE tensor_add -> uint16(round(s_scaled + ebp)), where
        ebp = fp16(A16*b + 16256 + sigma): Schraudolph fast-exp with the
        bias-add folded into the same instruction.
     P/V: ACT exp(s_scaled/A16) -> bf16 es; es * ebs (bf16 exp(b)) on
        Pool (P) or DVE 2x (V).
 - attend per (head, chunk, q-block): acc[128 Q, 33] += pr^T . [v_h | 2.0],
   accumulated over 16 K-chunks; col 32 = 2*sum(w) (denominator).
 - tail per head: recip, gr = (1+tanh)*recip (sigmoid via tanh, bg added by a
   ones-row matmul in the g-projection), og = acc*gr -> bf16 [Q, HD].
 - PE transposes og -> oT; output projection in [Q, C] f32; bo added on host.
"""

import math

import numpy as np
import ml_dtypes

B, Q, K = 2, 2048, 2048
C = 256
H, D = 8, 32
HD = H * D
QS = Q // 4
NCORES = 8

A16 = 128.0 * math.log2(math.e)          # 184.664...
SIGMA = -4.7
EB_CONST = 16256.0 + SIGMA               # 127<<7 + schraudolph centering


# routing per chunk c (0..15), uniform over heads; types interleaved so
# consecutive rounds hit different engines:
# 'S' = DVE schraudolph-add; 'P' = ACT exp + Pool mult; 'V' = ACT + DVE mult
ROUTE_C = ["S", "P", "S", "V", "P", "S", "V", "P",
           "S", "P", "S", "P", "S", "V", "S", "P"]


def _route(h, c):
    return ROUTE_C[c]


S_CS = [c for c in range(16) if ROUTE_C[c] == "S"]
A_CS = [c for c in range(16) if ROUTE_C[c] != "S"]
EBP_SLOT = {c: i for i, c in enumerate(S_CS)}   # chunk -> ebp slot
EBS_SLOT = {c: i for i, c in enumerate(A_CS)}   # chunk -> ebs slot
N_EBP = len(S_CS)
N_EBS = len(A_CS)

_CACHE = {}


def _build_nc():
    import concourse.mybir as mybir
    import concourse.tile as tile
    from concourse import bacc
    import concourse.bass as bass

    F32 = mybir.dt.float32
    F16 = mybir.dt.float16
    BF16 = mybir.dt.bfloat16
    U16 = mybir.dt.uint16
    F8 = mybir.dt.float8e4
    EXPF = mybir.ActivationFunctionType.Exp
    TANH = mybir.ActivationFunctionType.Tanh
    MUL = mybir.AluOpType.mult
    ADD = mybir.AluOpType.add
    DR = mybir.MatmulPerfMode.DoubleRow
    AP = bass.AP

    nc = bacc.Bacc("TRN2", target_bir_lowering=False, debug=False,
                   num_devices=NCORES)

    def din(name, shape, dt):
        return nc.declare_dram_parameter(name, shape, dt, isOutput=False).ap()

    kvD = din("kv", [C, K], BF16)
    wqkD = din("wqk", [C, 512], BF16)     # wq_eff.T t-halves | Wk.T
    blobD = din("blob", [C, 1280], BF16)  # qxT 512 | wvT+wgT 512 | woT 256
    bgD = din("bg", [1, C], BF16)
    idD = din("id", [128, 128], BF16)
    ebpD = din("ebp", [N_EBP * 128, QS], F16)
    ebsD = din("ebs", [N_EBS * 128, QS], BF16)
    outD = nc.declare_dram_parameter("out", [QS, C], F32, isOutput=True).ap()

    def ap3(t, dims, offset=0):
        # free-dim reshape of a tile/AP -> AP with dims [(stride, n), ...]
        return AP(tensor=t.tensor, offset=t.offset + offset,
                  ap=[list(t.ap[0])] + [[s, n] for s, n in dims])

    def dhalf(apD, X):
        # dram [256, X] -> AP matching sbuf [128 p, 2 half, X]: row = p+128*half
        return AP(tensor=apD.tensor, offset=apD.offset,
                  ap=[[X, 128], [128 * X, 2], [1, X]])

    def dchunk(apD, n, X):
        # dram [n*128, X] -> AP matching sbuf [128 p, n c, X]: row = c*128+p
        return AP(tensor=apD.tensor, offset=apD.offset,
                  ap=[[X, 128], [128 * X, n], [1, X]])

    from contextlib import ExitStack
    with tile.TileContext(nc) as tc:
        with tc.tile_pool(name="wp", bufs=1) as wp, \
             tc.tile_pool(name="dp", bufs=1) as dp, \
             tc.tile_pool(name="prp", bufs=7) as prp, \
             tc.tile_pool(name="esp", bufs=4) as esp, \
             tc.tile_pool(name="osp", bufs=2) as osp, \
             ExitStack() as stk:

            mm = nc.tensor.matmul

            # ---------------- input DMAs ----------------
            kv = wp.tile([128, 2, K], BF16, tag="kv", name="kv")
            nc.sync.dma_start(out=kv, in_=dhalf(kvD, K))
            wqk = wp.tile([128, 2, 512], BF16, tag="wqk", name="wqk")
            nc.sync.dma_start(out=wqk, in_=dhalf(wqkD, 512))
            blob = wp.tile([128, 2, 1280], BF16, tag="blob", name="blob")
            nc.sync.dma_start(out=blob, in_=dhalf(blobD, 1280))
            ident = wp.tile([128, 128], BF16, tag="id", name="id")
            nc.scalar.dma_start(out=ident, in_=idD)
            bgr = wp.tile([1, C], BF16, tag="bgr", name="bgr")
            nc.scalar.dma_start(out=bgr, in_=bgD)
            ebp = wp.tile([128, N_EBP, QS], F16, tag="ebp", name="ebp")
            ebs = wp.tile([128, N_EBS, QS], BF16, tag="ebs", name="ebs")
            # split bias DMAs so early chunks land early
            np_half = N_EBP // 2
            ns_half = N_EBS // 2
            ebpA = dchunk(ebpD, N_EBP, QS)
            ebsA = dchunk(ebsD, N_EBS, QS)
            nc.sync.dma_start(out=ebp[:, 0:np_half, :],
                              in_=ebpA[:, 0:np_half, :])
            nc.scalar.dma_start(out=ebs[:, 0:ns_half, :],
                                in_=ebsA[:, 0:ns_half, :])
            nc.sync.dma_start(out=ebp[:, np_half:N_EBP, :],
                              in_=ebpA[:, np_half:N_EBP, :])
            nc.scalar.dma_start(out=ebs[:, ns_half:N_EBS, :],
                                in_=ebsA[:, ns_half:N_EBS, :])

            zeros = wp.tile([128, 1], F32, tag="zeros", name="zeros")
            nc.vector.memset(zeros, 0.0)
            warm = wp.tile([128, 512], BF16, tag="warm", name="warm")
            nc.gpsimd.memset(warm, 0.0)
            ones = wp.tile([1, 128], BF16, tag="ones", name="ones")
            nc.gpsimd.memset(ones, 1.0)

            # persistent activations
            qT = dp.tile([128, 2, QS], BF16, tag="qT", name="qT")
            kT = dp.tile([128, 2, K], BF16, tag="kT", name="kT")
            v8 = dp.tile([128, 16, 8 * 33], BF16, tag="v8", name="v8")
            gth = dp.tile([128, 4, HD], BF16, tag="gth", name="gth")
            og = dp.tile([128, 4, HD], BF16, tag="og", name="og")
            oT = dp.tile([128, 2, QS], BF16, tag="oT", name="oT")

            # v8 denominator columns
            nc.gpsimd.memset(ap3(v8, [(264, 16), (33, 8)], offset=32), 2.0)

            # ---------------- projections ----------------
            pproj_cm = tc.tile_pool(name="pproj", bufs=4, space="PSUM")
            pproj = pproj_cm.__enter__()

            # PE pstate warm-up: dummy matmuls into one psum buffer while
            # input DMAs are in flight (sets pe_busy_start early)
            wps = pproj.tile([128, 512], F32, tag="warmp", name="warmp")
            for i in range(8):
                mm(wps, warm[:, 0:128], warm,
                   start=True, stop=True, skip_group_check=True)

            def emit_qT():
                for t in range(2):
                    pp = pproj.tile([128, QS], F32, tag="pp", name=f"ppq{t}")
                    mm(pp, wqk[:, 0, 128 * t:128 * (t + 1)],
                       blob[:, 0, 0:512], start=True, stop=False)
                    mm(pp, wqk[:, 1, 128 * t:128 * (t + 1)],
                       blob[:, 1, 0:512], start=False, stop=True)
                    nc.scalar.copy(qT[:, t, :], pp)

            def emit_kT(kb):
                for t in range(2):
                    pp = pproj.tile([128, QS], F32, tag="pp",
                                    name=f"ppk{kb}{t}")
                    sl = slice(512 * kb, 512 * (kb + 1))
                    mm(pp, wqk[:, 0, 256 + 128 * t:256 + 128 * (t + 1)],
                       kv[:, 0, sl], start=True, stop=False)
                    mm(pp, wqk[:, 1, 256 + 128 * t:256 + 128 * (t + 1)],
                       kv[:, 1, sl], start=False, stop=True)
                    if (kb + t) % 2 == 0:
                        nc.scalar.copy(kT[:, t, sl], pp)
                    else:
                        nc.vector.tensor_copy(kT[:, t, sl], pp)

            def emit_v(r):
                # chunks 2r, 2r+1 -> one [128, 512] psum
                pp = pproj.tile([128, 512], F32, tag="pp", name=f"ppv{r}")
                for j in range(2):
                    c = 2 * r + j
                    ksl = slice(128 * c, 128 * (c + 1))
                    mm(pp[:, 256 * j:256 * (j + 1)],
                       kv[:, 0, ksl], blob[:, 0, 512:768], start=True, stop=False)
                    mm(pp[:, 256 * j:256 * (j + 1)],
                       kv[:, 1, ksl], blob[:, 1, 512:768], start=False, stop=True)
                dst = ap3(v8, [(264, 2), (33, 8), (1, 32)], offset=264 * 2 * r)
                src = ap3(pp, [(256, 2), (32, 8), (1, 32)])
                if r % 2 == 0:
                    nc.scalar.copy(dst, src)
                else:
                    nc.vector.tensor_copy(dst, src)

            emit_kT(0)
            emit_kT(1)
            emit_qT()
            emit_v(0)
            emit_kT(2)
            emit_v(1)
            emit_kT(3)
            emit_v(2)
            emit_v(3)
            for r in range(4, 8):
                emit_v(r)

            # gating projection -> tanh((x+bg)/2), layout [Q part, HD free]
            for qb in range(4):
                pg = pproj.tile([128, HD], F32, tag="pp", name=f"ppg{qb}")
                qsl = slice(128 * qb, 128 * (qb + 1))
                mm(pg, blob[:, 0, qsl], blob[:, 0, 768:1024],
                   start=True, stop=False)
                mm(pg, blob[:, 1, qsl], blob[:, 1, 768:1024],
                   start=False, stop=False)
                mm(pg, ones[:, 0:128], bgr, start=False, stop=True,
                   tile_position=(0, 0))
                nc.scalar.activation(gth[:, qb, :], pg, TANH,
                                     bias=zeros, scale=0.5)

            pproj_cm.__exit__(None, None, None)
            pq = stk.enter_context(
                tc.tile_pool(name="pq", bufs=6, space="PSUM"))
            pacc = stk.enter_context(
                tc.tile_pool(name="pacc", bufs=2, space="PSUM"))

            # ---------------- main rounds ----------------
            # software-pipelined emission: scores(r) | route(r-1) | attend(r-2)
            rounds = [(h, c) for h in range(8) for c in range(16)]
            quads = {}
            prs = {}
            accs = {}

            def emit_scores(r):
                h, c = rounds[r]
                ht, hh = h // 4, h % 4
                psl = slice(32 * hh, 32 * (hh + 1))
                quad = pq.tile([128, 512], F32, tag="quad", name=f"qd{h}_{c}")
                mm(quad, kT[psl, ht, 128 * c:128 * (c + 1)], qT[psl, ht, :],
                   start=True, stop=True, tile_position=(32 * hh, 0))
                quads[r] = quad

            def emit_route(r):
                h, c = rounds[r]
                quad = quads.pop(r)
                route = _route(h, c)
                pr = prp.tile([128, 512], U16, tag="pr", name=f"pr{h}_{c}")
                if route == "S":
                    nc.vector.scalar_tensor_tensor(
                        out=pr, in0=quad, scalar=EB_CONST,
                        in1=ebp[:, EBP_SLOT[c], :],
                        op0=ADD, op1=ADD)
                else:
                    es = esp.tile([128, 512], BF16, tag="es",
                                  name=f"es{h}_{c}")
                    nc.scalar.activation(es, quad, EXPF,
                                         bias=zeros, scale=1.0 / A16)
                    eng = nc.gpsimd if route == "P" else nc.vector
                    eng.tensor_mul(pr.bitcast(BF16), es,
                                   ebs[:, EBS_SLOT[c], :])
                prs[r] = pr

            def emit_attend(r):
                h, c = rounds[r]
                if c == 0:
                    accs[h] = pacc.tile([128, 512], F32, tag="acc",
                                        name=f"acc{h}")
                    # zero the whole bank once; all attends accumulate
                    mm(accs[h], warm[0:1, 0:128], warm[0:1, :],
                       start=True, stop=False, skip_group_check=True,
                       tile_position=(0, 0))
                acc = accs[h]
                prb = prs.pop(r).bitcast(BF16)
                for qb in range(4):
                    mm(acc[:, 128 * qb:128 * qb + 33],
                       prb[:, 128 * qb:128 * (qb + 1)],
                       ap3(v8, [(1, 33)], offset=264 * c + 33 * h),
                       start=False, stop=(c == 15), skip_group_check=True)
            def emit_tail(h):
                acc = accs.pop(h)
                rec = osp.tile([128, 4], F32, tag="rec", name=f"rec{h}")
                nc.vector.reciprocal(rec, ap3(acc, [(128, 4)], offset=32))
                grt = osp.tile([128, 4, 32], F32, tag="gr", name=f"gr{h}")
                nc.vector.scalar_tensor_tensor(
                    out=grt,
                    in0=ap3(gth, [(HD, 4), (1, 32)], offset=32 * h),
                    scalar=1.0,
                    in1=ap3(rec, [(1, 4), (0, 32)]),
                    op0=ADD, op1=MUL)
                nc.vector.tensor_mul(
                    ap3(og, [(HD, 4), (1, 32)], offset=32 * h),
                    ap3(acc, [(128, 4), (1, 32)]),
                    grt)

            LAG = 4
            TAILLAG = 2
            for r in range(len(rounds) + LAG + TAILLAG):
                if r < len(rounds):
                    emit_scores(r)
                if 1 <= r < len(rounds) + 1:
                    emit_route(r - 1)
                if LAG <= r < len(rounds) + LAG:
                    emit_attend(r - LAG)
                rt = r - LAG - TAILLAG
                if rt >= 0 and rt % 16 == 15:
                    emit_tail(rt // 16)

            # ---------------- output ----------------
            for qb in range(4):
                for half in range(2):
                    tr = pq.tile([128, 128], BF16, tag="quad",
                                 name=f"tr{qb}{half}")
                    nc.tensor.transpose(
                        tr, ap3(og, [(1, 128)], offset=HD * qb + 128 * half),
                        ident)
                    if half == 0:
                        nc.vector.tensor_copy(
                            oT[:, half, 128 * qb:128 * (qb + 1)], tr)
                    else:
                        nc.scalar.copy(
                            oT[:, half, 128 * qb:128 * (qb + 1)], tr)
                fin = pq.tile([128, C], F32, tag="quad", name=f"fin{qb}")
                mm(fin, oT[:, 0, 128 * qb:128 * (qb + 1)], blob[:, 0, 1024:1280],
                   start=True, stop=False)
                mm(fin, oT[:, 1, 128 * qb:128 * (qb + 1)], blob[:, 1, 1024:1280],
                   start=False, stop=True)
                osb = osp.tile([128, C], F32, tag="osb", name=f"osb{qb}")
                nc.scalar.copy(osb, fin)
                nc.sync.dma_start(out=outD[128 * qb:128 * (qb + 1), :],
                                  in_=osb)

    nc.compile()
    return nc


def _host_inputs(q_x, kv_x, bias, Wq, Wk, Wv, Wo, bo, Wg, bg):
    f32 = np.float32
    bf = ml_dtypes.bfloat16
    wq_eff = (np.asarray(Wq, f32) * (A16 / math.sqrt(D)))
    wqk = np.concatenate([wq_eff.T, np.asarray(Wk, f32).T], axis=1)  # [C, 512]
    shared = {
        "wqk": wqk.astype(bf),
        "bg": np.asarray(bg, f32).reshape(1, C).astype(bf),
        "id": np.eye(128, dtype=bf),
    }
    kvT = [np.ascontiguousarray(kv_x[b].T).astype(bf) for b in range(B)]
    in_maps = []
    for core in range(NCORES):
        b, qc = core // 4, core % 4
        rows = slice(QS * qc, QS * (qc + 1))
        bT = np.ascontiguousarray(bias[b, 0, rows, :].T).astype(f32)  # [K, QS]
        ebp = np.concatenate(
            [A16 * bT[128 * c:128 * (c + 1), :] for c in S_CS],
            axis=0).astype(np.float16)
        ebs = np.concatenate(
            [np.exp(bT[128 * c:128 * (c + 1), :]) for c in A_CS],
            axis=0).astype(bf)
        m = dict(shared)
        qxT = np.ascontiguousarray(q_x[b, rows, :].T).astype(f32)
        blob = np.concatenate(
            [qxT, Wv.T, Wg.T, Wo.T.astype(f32)], axis=1)     # [C, 1280]
        m["blob"] = blob.astype(bf)
        m["kv"] = kvT[b]
        m["ebp"] = ebp
        m["ebs"] = ebs
        in_maps.append(m)
    return in_maps


def kernel(q_x, kv_x, bias, Wq, Wk, Wv, Wo, bo, Wg, bg, _profile=False):
    from concourse.bass_utils import run_bass_kernel_spmd

    q_x = np.asarray(q_x, dtype=np.float32)
    kv_x = np.asarray(kv_x, dtype=np.float32)
    bias = np.asarray(bias, dtype=np.float32)

    if "nc" not in _CACHE:
        _CACHE["nc"] = _build_nc()
    nc = _CACHE["nc"]

    in_maps = _host_inputs(q_x, kv_x, bias,
                           np.asarray(Wq, np.float32),
                           np.asarray(Wk, np.float32),
                           np.asarray(Wv, np.float32),
                           np.asarray(Wo, np.float32),
                           np.asarray(bo, np.float32),
                           np.asarray(Wg, np.float32),
                           np.asarray(bg, np.float32))

    res = run_bass_kernel_spmd(nc, in_maps, list(range(NCORES)),
                               trace=_profile)
    out = np.empty((B, Q, C), dtype=np.float32)
    bo32 = np.asarray(bo, np.float32)
    for core in range(NCORES):
        b, qc = core // 4, core % 4
        out[b, QS * qc:QS * (qc + 1), :] = res.results[core]["out"] + bo32
    if _profile:
        _CACHE["last_exec_time_ns"] = res.exec_time_ns
        _CACHE["last_results"] = res
    return out


# revision 24
# speedup vs baseline: 1.0306x; 1.0306x over previous
"""Gated multi-head attention (AlphaFold-style) on 8 Trainium2 NeuronCores.

Sharding: 8 cores = 2 batches x 4 query-chunks of 512 rows; each core does all
8 heads for its (b, q-chunk); outputs are disjoint row blocks (no collectives).

Per-core pipeline:
 - q/k projected to fp8e4m3, DoubleRow-packed: partition 32g+dd holds head
   2g+t dim dd on k-tile t.  kT8 carries both heads of a pair; the q side is
   split into qT8e/qT8o with zeros in the partner head's k-tile slots, so one
   DR matmul (0.5 cyc/row) contracts exactly one head's 32 dims and emits
   s_scaled = A16*s (A16 = 128*log2(e) folded into Wq host-side).
 - softmax weights w = exp(s+b) as bf16 BIT PATTERNS, three routes balancing
   ACT/DVE/Pool:
     S: one DVE tensor_add -> uint16(round(s_scaled + ebp)), where
        ebp = fp16(A16*b + 16256 + sigma): Schraudolph fast-exp with the
        bias-add folded into the same instruction.
     P/V: ACT exp(s_scaled/A16) -> bf16 es; es * ebs (bf16 exp(b)) on
        Pool (P) or DVE 2x (V).
 - attend per (head, chunk, q-block): acc[128 Q, 33] += pr^T . [v_h | 2.0],
   accumulated over 16 K-chunks; col 32 = 2*sum(w) (denominator).
 - tail per head: recip, gr = (1+tanh)*recip (sigmoid via tanh, bg added by a
   ones-row matmul in the g-projection), og = acc*gr -> bf16 [Q, HD].
 - PE transposes og -> oT; output projection in [Q, C] f32; bo added on host.
"""

import math

import numpy as np
import ml_dtypes

B, Q, K = 2, 2048, 2048
C = 256
H, D = 8, 32
HD = H * D
QS = Q // 4
NCORES = 8

A16 = 128.0 * math.log2(math.e)          # 184.664...
SIGMA = -4.7
EB_CONST = 16256.0 + SIGMA               # 127<<7 + schraudolph centering


# routing per chunk c (0..15), uniform over heads; types interleaved so
# consecutive rounds hit different engines:
# 'S' = DVE schraudolph-add; 'P' = ACT exp + Pool mult; 'V' = ACT + DVE mult
ROUTE_C = ["S", "P", "S", "V", "P", "S", "V", "P",
           "S", "V", "S", "P", "S", "V", "S", "P"]


def _route(h, c):
    return ROUTE_C[c]


S_CS = [c for c in range(16) if ROUTE_C[c] == "S"]
A_CS = [c for c in range(16) if ROUTE_C[c] != "S"]
EBP_SLOT = {c: i for i, c in enumerate(S_CS)}   # chunk -> ebp slot
EBS_SLOT = {c: i for i, c in enumerate(A_CS)}   # chunk -> ebs slot
N_EBP = len(S_CS)
N_EBS = len(A_CS)

_CACHE = {}


def _build_nc():
    import concourse.mybir as mybir
    import concourse.tile as tile
    from concourse import bacc
    import concourse.bass as bass

    F32 = mybir.dt.float32
    F16 = mybir.dt.float16
    BF16 = mybir.dt.bfloat16
    U16 = mybir.dt.uint16
    F8 = mybir.dt.float8e4
    EXPF = mybir.ActivationFunctionType.Exp
    TANH = mybir.ActivationFunctionType.Tanh
    MUL = mybir.AluOpType.mult
    ADD = mybir.AluOpType.add
    DR = mybir.MatmulPerfMode.DoubleRow
    AP = bass.AP

    nc = bacc.Bacc("TRN2", target_bir_lowering=False, debug=False,
                   num_devices=NCORES)

    def din(name, shape, dt):
        return nc.declare_dram_parameter(name, shape, dt, isOutput=False).ap()

    kvD = din("kv", [C, K], BF16)
    wqkD = din("wqk", [C, 512], BF16)     # wq_eff.T t-halves | Wk.T
    blobD = din("blob", [C, 1280], BF16)  # qxT 512 | wvT+wgT 512 | woT 256
    bgD = din("bg", [1, C], BF16)
    idD = din("id", [128, 128], BF16)
    ebpD = din("ebp", [N_EBP * 128, QS], F16)
    ebsD = din("ebs", [N_EBS * 128, QS], BF16)
    outD = nc.declare_dram_parameter("out", [QS, C], F32, isOutput=True).ap()

    def ap3(t, dims, offset=0):
        # free-dim reshape of a tile/AP -> AP with dims [(stride, n), ...]
        return AP(tensor=t.tensor, offset=t.offset + offset,
                  ap=[list(t.ap[0])] + [[s, n] for s, n in dims])

    def dhalf(apD, X):
        # dram [256, X] -> AP matching sbuf [128 p, 2 half, X]: row = p+128*half
        return AP(tensor=apD.tensor, offset=apD.offset,
                  ap=[[X, 128], [128 * X, 2], [1, X]])

    def dchunk(apD, n, X):
        # dram [n*128, X] -> AP matching sbuf [128 p, n c, X]: row = c*128+p
        return AP(tensor=apD.tensor, offset=apD.offset,
                  ap=[[X, 128], [128 * X, n], [1, X]])

    from contextlib import ExitStack
    with tile.TileContext(nc) as tc:
        with tc.tile_pool(name="wp", bufs=1) as wp, \
             tc.tile_pool(name="dp", bufs=1) as dp, \
             tc.tile_pool(name="prp", bufs=7) as prp, \
             tc.tile_pool(name="esp", bufs=4) as esp, \
             tc.tile_pool(name="osp", bufs=2) as osp, \
             ExitStack() as stk:

            mm = nc.tensor.matmul

            # ---------------- input DMAs ----------------
            kv = wp.tile([128, 2, K], BF16, tag="kv", name="kv")
            nc.sync.dma_start(out=kv, in_=dhalf(kvD, K))
            wqk = wp.tile([128, 2, 512], BF16, tag="wqk", name="wqk")
            nc.sync.dma_start(out=wqk, in_=dhalf(wqkD, 512))
            blob = wp.tile([128, 2, 1280], BF16, tag="blob", name="blob")
            nc.sync.dma_start(out=blob, in_=dhalf(blobD, 1280))
            ident = wp.tile([128, 128], BF16, tag="id", name="id")
            nc.scalar.dma_start(out=ident, in_=idD)
            bgr = wp.tile([1, C], BF16, tag="bgr", name="bgr")
            nc.scalar.dma_start(out=bgr, in_=bgD)
            ebp = wp.tile([128, N_EBP, QS], F16, tag="ebp", name="ebp")
            ebs = wp.tile([128, N_EBS, QS], BF16, tag="ebs", name="ebs")
            # split bias DMAs so early chunks land early
            np_half = N_EBP // 2
            ns_half = N_EBS // 2
            ebpA = dchunk(ebpD, N_EBP, QS)
            ebsA = dchunk(ebsD, N_EBS, QS)
            nc.sync.dma_start(out=ebp[:, 0:np_half, :],
                              in_=ebpA[:, 0:np_half, :])
            nc.scalar.dma_start(out=ebs[:, 0:ns_half, :],
                                in_=ebsA[:, 0:ns_half, :])
            nc.sync.dma_start(out=ebp[:, np_half:N_EBP, :],
                              in_=ebpA[:, np_half:N_EBP, :])
            nc.scalar.dma_start(out=ebs[:, ns_half:N_EBS, :],
                                in_=ebsA[:, ns_half:N_EBS, :])

            zeros = wp.tile([128, 1], F32, tag="zeros", name="zeros")
            nc.vector.memset(zeros, 0.0)
            warm = wp.tile([128, 512], BF16, tag="warm", name="warm")
            nc.gpsimd.memset(warm, 0.0)
            ones = wp.tile([1, 128], BF16, tag="ones", name="ones")
            nc.gpsimd.memset(ones, 1.0)

            # persistent activations
            qT = dp.tile([128, 2, QS], BF16, tag="qT", name="qT")
            kT = dp.tile([128, 2, K], BF16, tag="kT", name="kT")
            v8 = dp.tile([128, 16, 8 * 33], BF16, tag="v8", name="v8")
            gth = dp.tile([128, 4, HD], BF16, tag="gth", name="gth")
            og = dp.tile([128, 4, HD], BF16, tag="og", name="og")
            oT = dp.tile([128, 2, QS], BF16, tag="oT", name="oT")

            # v8 denominator columns
            nc.gpsimd.memset(ap3(v8, [(264, 16), (33, 8)], offset=32), 2.0)

            # ---------------- projections ----------------
            pproj_cm = tc.tile_pool(name="pproj", bufs=4, space="PSUM")
            pproj = pproj_cm.__enter__()

            # PE pstate warm-up: dummy matmuls into one psum buffer while
            # input DMAs are in flight (sets pe_busy_start early)
            wps = pproj.tile([128, 512], F32, tag="warmp", name="warmp")
            for i in range(8):
                mm(wps, warm[:, 0:128], warm,
                   start=True, stop=True, skip_group_check=True)

            def emit_qT():
                for t in range(2):
                    pp = pproj.tile([128, QS], F32, tag="pp", name=f"ppq{t}")
                    mm(pp, wqk[:, 0, 128 * t:128 * (t + 1)],
                       blob[:, 0, 0:512], start=True, stop=False)
                    mm(pp, wqk[:, 1, 128 * t:128 * (t + 1)],
                       blob[:, 1, 0:512], start=False, stop=True)
                    nc.scalar.copy(qT[:, t, :], pp)

            def emit_kT(kb):
                for t in range(2):
                    pp = pproj.tile([128, QS], F32, tag="pp",
                                    name=f"ppk{kb}{t}")
                    sl = slice(512 * kb, 512 * (kb + 1))
                    mm(pp, wqk[:, 0, 256 + 128 * t:256 + 128 * (t + 1)],
                       kv[:, 0, sl], start=True, stop=False)
                    mm(pp, wqk[:, 1, 256 + 128 * t:256 + 128 * (t + 1)],
                       kv[:, 1, sl], start=False, stop=True)
                    if (kb + t) % 2 == 0:
                        nc.scalar.copy(kT[:, t, sl], pp)
                    else:
                        nc.vector.tensor_copy(kT[:, t, sl], pp)

            def emit_v(r):
                # chunks 2r, 2r+1 -> one [128, 512] psum
                pp = pproj.tile([128, 512], F32, tag="pp", name=f"ppv{r}")
                for j in range(2):
                    c = 2 * r + j
                    ksl = slice(128 * c, 128 * (c + 1))
                    mm(pp[:, 256 * j:256 * (j + 1)],
                       kv[:, 0, ksl], blob[:, 0, 512:768], start=True, stop=False)
                    mm(pp[:, 256 * j:256 * (j + 1)],
                       kv[:, 1, ksl], blob[:, 1, 512:768], start=False, stop=True)
                dst = ap3(v8, [(264, 2), (33, 8), (1, 32)], offset=264 * 2 * r)
                src = ap3(pp, [(256, 2), (32, 8), (1, 32)])
                if r % 2 == 0:
                    nc.scalar.copy(dst, src)
                else:
                    nc.vector.tensor_copy(dst, src)

            emit_kT(0)
            emit_kT(1)
            emit_qT()
            emit_v(0)
            emit_kT(2)
            emit_v(1)
            emit_kT(3)
            emit_v(2)
            emit_v(3)
            for r in range(4, 8):
                emit_v(r)

            # gating projection -> tanh((x+bg)/2), layout [Q part, HD free]
            for qb in range(4):
                pg = pproj.tile([128, HD], F32, tag="pp", name=f"ppg{qb}")
                qsl = slice(128 * qb, 128 * (qb + 1))
                mm(pg, blob[:, 0, qsl], blob[:, 0, 768:1024],
                   start=True, stop=False)
                mm(pg, blob[:, 1, qsl], blob[:, 1, 768:1024],
                   start=False, stop=False)
                mm(pg, ones[:, 0:128], bgr, start=False, stop=True,
                   tile_position=(0, 0))
                nc.scalar.activation(gth[:, qb, :], pg, TANH,
                                     bias=zeros, scale=0.5)

            pproj_cm.__exit__(None, None, None)
            pq = stk.enter_context(
                tc.tile_pool(name="pq", bufs=6, space="PSUM"))
            pacc = stk.enter_context(
                tc.tile_pool(name="pacc", bufs=2, space="PSUM"))

            # ---------------- main rounds ----------------
            # software-pipelined emission: scores(r) | route(r-1) | attend(r-2)
            rounds = [(h, c) for h in range(8) for c in range(16)]
            quads = {}
            prs = {}
            accs = {}

            def emit_scores(r):
                h, c = rounds[r]
                ht, hh = h // 4, h % 4
                psl = slice(32 * hh, 32 * (hh + 1))
                quad = pq.tile([128, 512], F32, tag="quad", name=f"qd{h}_{c}")
                mm(quad, kT[psl, ht, 128 * c:128 * (c + 1)], qT[psl, ht, :],
                   start=True, stop=True, tile_position=(32 * hh, 0))
                quads[r] = quad

            def emit_route(r):
                h, c = rounds[r]
                quad = quads.pop(r)
                route = _route(h, c)
                pr = prp.tile([128, 512], U16, tag="pr", name=f"pr{h}_{c}")
                if route == "S":
                    nc.vector.scalar_tensor_tensor(
                        out=pr, in0=quad, scalar=EB_CONST,
                        in1=ebp[:, EBP_SLOT[c], :],
                        op0=ADD, op1=ADD)
                else:
                    es = esp.tile([128, 512], BF16, tag="es",
                                  name=f"es{h}_{c}")
                    nc.scalar.activation(es, quad, EXPF,
                                         bias=zeros, scale=1.0 / A16)
                    eng = nc.gpsimd if route == "P" else nc.vector
                    eng.tensor_mul(pr.bitcast(BF16), es,
                                   ebs[:, EBS_SLOT[c], :])
                prs[r] = pr

            def emit_attend(r):
                h, c = rounds[r]
                if c == 0:
                    accs[h] = pacc.tile([128, 512], F32, tag="acc",
                                        name=f"acc{h}")
                    # zero the whole bank once; all attends accumulate
                    mm(accs[h], warm[0:1, 0:128], warm[0:1, :],
                       start=True, stop=False, skip_group_check=True,
                       tile_position=(0, 0))
                acc = accs[h]
                prb = prs.pop(r).bitcast(BF16)
                for qb in range(4):
                    mm(acc[:, 128 * qb:128 * qb + 33],
                       prb[:, 128 * qb:128 * (qb + 1)],
                       ap3(v8, [(1, 33)], offset=264 * c + 33 * h),
                       start=False, stop=(c == 15), skip_group_check=True)
            def emit_tail(h):
                acc = accs.pop(h)
                rec = osp.tile([128, 4], F32, tag="rec", name=f"rec{h}")
                nc.vector.reciprocal(rec, ap3(acc, [(128, 4)], offset=32))
                grt = osp.tile([128, 4, 32], F32, tag="gr", name=f"gr{h}")
                nc.vector.scalar_tensor_tensor(
                    out=grt,
                    in0=ap3(gth, [(HD, 4), (1, 32)], offset=32 * h),
                    scalar=1.0,
                    in1=ap3(rec, [(1, 4), (0, 32)]),
                    op0=ADD, op1=MUL)
                nc.vector.tensor_mul(
                    ap3(og, [(HD, 4), (1, 32)], offset=32 * h),
                    ap3(acc, [(128, 4), (1, 32)]),
                    grt)

            LAG = 4
            TAILLAG = 2
            for r in range(len(rounds) + LAG + TAILLAG):
                if r < len(rounds):
                    emit_scores(r)
                if 1 <= r < len(rounds) + 1:
                    emit_route(r - 1)
                if LAG <= r < len(rounds) + LAG:
                    emit_attend(r - LAG)
                rt = r - LAG - TAILLAG
                if rt >= 0 and rt % 16 == 15:
                    emit_tail(rt // 16)

            # ---------------- output ----------------
            for qb in range(4):
                for half in range(2):
                    tr = pq.tile([128, 128], BF16, tag="quad",
                                 name=f"tr{qb}{half}")
                    nc.tensor.transpose(
                        tr, ap3(og, [(1, 128)], offset=HD * qb + 128 * half),
                        ident)
                    if half == 0:
                        nc.vector.tensor_copy(
                            oT[:, half, 128 * qb:128 * (qb + 1)], tr)
                    else:
                        nc.scalar.copy(
                            oT[:, half, 128 * qb:128 * (qb + 1)], tr)
                fin = pq.tile([128, C], F32, tag="quad", name=f"fin{qb}")
                mm(fin, oT[:, 0, 128 * qb:128 * (qb + 1)], blob[:, 0, 1024:1280],
                   start=True, stop=False)
                mm(fin, oT[:, 1, 128 * qb:128 * (qb + 1)], blob[:, 1, 1024:1280],
                   start=False, stop=True)
                osb = osp.tile([128, C], F32, tag="osb", name=f"osb{qb}")
                nc.scalar.copy(osb, fin)
                nc.sync.dma_start(out=outD[128 * qb:128 * (qb + 1), :],
                                  in_=osb)

    nc.compile()
    return nc


def _host_inputs(q_x, kv_x, bias, Wq, Wk, Wv, Wo, bo, Wg, bg):
    f32 = np.float32
    bf = ml_dtypes.bfloat16
    wq_eff = (np.asarray(Wq, f32) * (A16 / math.sqrt(D)))
    wqk = np.concatenate([wq_eff.T, np.asarray(Wk, f32).T], axis=1)  # [C, 512]
    shared = {
        "wqk": wqk.astype(bf),
        "bg": np.asarray(bg, f32).reshape(1, C).astype(bf),
        "id": np.eye(128, dtype=bf),
    }
    kvT = [np.ascontiguousarray(kv_x[b].T).astype(bf) for b in range(B)]
    in_maps = []
    for core in range(NCORES):
        b, qc = core // 4, core % 4
        rows = slice(QS * qc, QS * (qc + 1))
        bT = np.ascontiguousarray(bias[b, 0, rows, :].T).astype(f32)  # [K, QS]
        ebp = np.concatenate(
            [A16 * bT[128 * c:128 * (c + 1), :] for c in S_CS],
            axis=0).astype(np.float16)
        ebs = np.concatenate(
            [np.exp(bT[128 * c:128 * (c + 1), :]) for c in A_CS],
            axis=0).astype(bf)
        m = dict(shared)
        qxT = np.ascontiguousarray(q_x[b, rows, :].T).astype(f32)
        blob = np.concatenate(
            [qxT, Wv.T, Wg.T, Wo.T.astype(f32)], axis=1)     # [C, 1280]
        m["blob"] = blob.astype(bf)
        m["kv"] = kvT[b]
        m["ebp"] = ebp
        m["ebs"] = ebs
        in_maps.append(m)
    return in_maps


def kernel(q_x, kv_x, bias, Wq, Wk, Wv, Wo, bo, Wg, bg, _profile=False):
    from concourse.bass_utils import run_bass_kernel_spmd

    q_x = np.asarray(q_x, dtype=np.float32)
    kv_x = np.asarray(kv_x, dtype=np.float32)
    bias = np.asarray(bias, dtype=np.float32)

    if "nc" not in _CACHE:
        _CACHE["nc"] = _build_nc()
    nc = _CACHE["nc"]

    in_maps = _host_inputs(q_x, kv_x, bias,
                           np.asarray(Wq, np.float32),
                           np.asarray(Wk, np.float32),
                           np.asarray(Wv, np.float32),
                           np.asarray(Wo, np.float32),
                           np.asarray(bo, np.float32),
                           np.asarray(Wg, np.float32),
                           np.asarray(bg, np.float32))

    res = run_bass_kernel_spmd(nc, in_maps, list(range(NCORES)),
                               trace=_profile)
    out = np.empty((B, Q, C), dtype=np.float32)
    bo32 = np.asarray(bo, np.float32)
    for core in range(NCORES):
        b, qc = core // 4, core % 4
        out[b, QS * qc:QS * (qc + 1), :] = res.results[core]["out"] + bo32
    if _profile:
        _CACHE["last_exec_time_ns"] = res.exec_time_ns
        _CACHE["last_results"] = res
    return out
